# revision 2
# baseline (speedup 1.0000x reference)
"""Trainium2 Bass kernel for nn_BertSVDBlock (B=8, M=1024, D=768, H=12).

Sharding: pure data-parallel over batch B - core b computes batch element b.

v3: everything fp32/float32r. TRN2 fp32r matmuls at N>=256 run at full
rate AND are self-loading (no separate LDWEIGHTS instruction), which
removes ~860 x 71ns of PE-sequencer issue overhead vs the bf16 design,
and improves precision. Other key points (all transposed layout [d|r, m]):
  - QKV first factors packed 4 heads/group, group order Q0 K0 V0 Q1 ...
    computed straight from xT (no bf16 x copy).
  - Q/K biases folded at PSUM evacuation (per-partition tensor_scalar);
    V bias + denominator-ones column via one broadcast TT add.
  - Q/K second factors head-pair packed; V second factors 4-heads/matmul.
  - Softmax denominators: DVE reciprocal, partition-broadcast via K=1
    fp32r ones-matmul into PSUM (no DRAM bounce).
  - LayerNorms: means folded out algebraically (colsum(x)/D early
    matmuls; host-precomputed colsum(Vo)/D, colsum(V2)/D broadcast
    stationaries; colsum(x1)=0). Stats as [128,M] broadcast tiles from
    ones-matmuls; variance from centered squares; rsqrt as
    exp(-0.5*ln(colsum_sq)+0.5*lnD) on ACT. LN1's per-column scale a1
    folded THROUGH the U1 matmul.
  - ~20 large DMAs total (the sim serializes DMA issue).
"""

import os
import sys

import numpy as np

import ml_dtypes

BF16 = ml_dtypes.bfloat16

for _p in ("/opt/trn_rl_repo", "/root/.axon_site/_ro/trn_rl_repo"):
    if os.path.isdir(_p) and _p not in sys.path:
        sys.path.append(_p)

B, M, D, H, DH = 8, 1024, 768, 12, 64
R_ATTN, R_FF, R_WO, DFF = 32, 256, 256, 3072
LN_EPS = 1e-12
N_CORES = 8
P = 128
KD = D // P            # 6 d-chunks
NPT = M // P           # 8 key chunks
FFT = DFF // P         # 24 dff chunks
NG = 9                 # 9 col groups in p_pack, order [Q K V] x 3
NPAIR = 6              # head pairs
VW = DH + 1            # 65: V columns + denominator-ones column
LOG_D = float(np.log(D))

# cpack column layout (per-partition fp32 consts)
C_MASK, C_BO, C_B2, C_B1, C_QKB = 0, 8, 14, 20, 44
C_TOT = 56
# w2pack column layout
W_Q, W_K, W_V = 0, NPAIR * P, 2 * NPAIR * P
W_TOT = 2 * NPAIR * P + 3 * 4 * VW     # 2316

_prog_cache: dict = {}
last_results = None


def _build_program(has_aff1: bool, has_aff2: bool,
                   bosum768: float, b2sum768: float):
    from contextlib import ExitStack

    import concourse.tile as tile
    from concourse import bacc
    from concourse import mybir

    f32 = mybir.dt.float32
    f32r = mybir.dt.float32r
    bf16 = mybir.dt.bfloat16
    AF = mybir.ActivationFunctionType
    OP = mybir.AluOpType

    nc = bacc.Bacc("TRN2", target_bir_lowering=False)

    def R(ap):
        return ap.bitcast(f32r)

    # ---- I/O (all fp32) ----
    xT_d = nc.dram_tensor("xT", [D, M], f32, kind="ExternalInput")
    pp_d = nc.dram_tensor("p_pack", [D, NG * P], bf16,
                          kind="ExternalInput")
    xb_d = nc.dram_tensor("xb", [D, M], bf16, kind="ExternalInput")
    w2_d = nc.dram_tensor("w2pack", [P, W_TOT], bf16,
                          kind="ExternalInput")
    cp_d = nc.dram_tensor("cpack", [P, C_TOT], f32, kind="ExternalInput")
    vbias_d = nc.dram_tensor("vbias", [1, 3 * 4 * VW], f32,
                             kind="ExternalInput")
    wp_d = nc.dram_tensor("wpack", [P, 2 * P], f32r,
                          kind="ExternalInput")
    wvo_d = nc.dram_tensor("wvopack", [P, 2 * P], bf16,
                           kind="ExternalInput")
    ov_d = nc.dram_tensor("ovpack", [P, KD * R_WO + 2 * D], bf16,
                          kind="ExternalInput")
    u1_d = nc.dram_tensor("u1", [D, R_FF], f32r, kind="ExternalInput")
    v1_d = nc.dram_tensor("v1", [R_FF, DFF], bf16, kind="ExternalInput")
    u2_d = nc.dram_tensor("u2", [DFF, R_FF], bf16, kind="ExternalInput")
    v2_d = nc.dram_tensor("v2", [R_FF, D], f32r, kind="ExternalInput")
    aff_d = {}
    if has_aff1 or has_aff2:
        aff_d["a"] = nc.dram_tensor("affpack", [P, 26], f32,
                                    kind="ExternalInput")
    out_d = nc.dram_tensor("outT", [D, M], bf16, kind="ExternalOutput")

    MI = (slice(0, 512), slice(512, 1024))

    with ExitStack() as top:
        # float32r outputs are fp32-width; the low-precision guard is about
        # 16-bit accumulation, which none of these ops do
        top.enter_context(nc.allow_low_precision(
            reason="float32r tiles are fp32-width"))
        tc = top.enter_context(tile.TileContext(nc))
        dma = nc.sync.dma_start
        mm = nc.tensor.matmul

        consts = top.enter_context(tc.tile_pool(name="consts", bufs=1))

        ones_f = consts.tile([P, P], f32r, name="ones_f")   # 1/D (bf16 MMs)
        nc.vector.memset(ones_f.bitcast(f32), 1.0 / D)
        ones_b16 = consts.tile([P, P], bf16, name="ones_b16")
        nc.vector.memset(ones_b16, 1.0 / D)
        ones_1 = consts.tile([P, P], f32r, name="ones_1")   # 1.0
        nc.vector.memset(ones_1.bitcast(f32), 1.0)
        half_lnD = consts.tile([P, 1], f32, name="half_lnD")
        nc.vector.memset(half_lnD, 0.5 * LOG_D)
        cpack = consts.tile([P, C_TOT], f32, name="cpack")
        vbias_b = consts.tile([P, 3 * 4 * VW], f32, name="vbias_b")
        wpack = consts.tile([P, 2 * P], f32r, name="wpack")
        wvow = consts.tile([P, 2 * P], bf16, name="wvow")
        affc = consts.tile([P, 26], f32, name="affc") if aff_d else None

        def cp(col, n=1):
            return cpack[:, col:col + n]

        maskb = lambda j: cp(C_MASK + j)          # noqa: E731
        boc = lambda k: cp(C_BO + k)              # noqa: E731
        b2c = lambda k: cp(C_B2 + k)              # noqa: E731
        b1c = lambda t: cp(C_B1 + t)              # noqa: E731
        qkbias = lambda i: cp(C_QKB + i)          # noqa: E731
        wvo_b = lambda r: wvow[:, r * P:(r + 1) * P]       # noqa: E731
        wv2_b = lambda r: wpack[:, r * P:(r + 1) * P]      # noqa: E731

        # persistent activation tiles
        keep = top.enter_context(tc.tile_pool(name="keep", bufs=1))
        xTt = keep.tile([P, KD, M], f32, name="xTt")
        xsum_s = keep.tile([P, M], f32, name="xsum_s")

        x1p = top.enter_context(tc.tile_pool(name="x1p", bufs=1))
        x1c = [x1p.tile([P, M], f32r, name=f"x1c{k}")
               for k in range(KD)]
        sqp = top.enter_context(tc.tile_pool(name="sqp", bufs=4))
        stats = top.enter_context(tc.tile_pool(name="stats", bufs=1))
        mu1_b = stats.tile([P, M], f32, name="mu1_b")
        a1_b = stats.tile([P, M], f32, name="a1_b")
        mu2_b = stats.tile([P, M], f32, name="mu2_b")
        a2_b = stats.tile([P, M], f32, name="a2_b")
        lnv = stats.tile([P, M], f32, name="lnv", tag="lnv", bufs=1)

        # PSUM: 2-slot pool reserved for score tiles (and later-phase
        # rotating psums); 1-slot aux pool for filler/broadcast psums
        ps_big = top.enter_context(
            tc.tile_pool(name="ps_big", bufs=2, space="PSUM"))

        def big_ps(name):
            return ps_big.tile([P, M], f32, tag="big", name=name)

        aux_ctx = top.enter_context(ExitStack())
        ps_aux = aux_ctx.enter_context(
            tc.tile_pool(name="ps_aux", bufs=1, space="PSUM"))
        state = {"use_aux": False}

        def aux_ps(name):
            if state["use_aux"]:
                return ps_aux.tile([P, M], f32, tag="aux", name=name)
            return big_ps(name)

        # outproj tensors (bf16: saves SBUF; ~52 extra LDWEIGHTS only)
        pB = top.enter_context(tc.tile_pool(name="pB", bufs=1))
        attn_sc = [pB.tile([P, M], bf16, name=f"attn_sc{k}")
                   for k in range(KD)]
        h1b = pB.tile([P, 2, M], bf16, name="h1b")
        ovw = pB.tile([P, KD * R_WO + 2 * D], bf16, name="ovw")

        def uo_ap(k, pt):
            return ovw[:, k * R_WO + pt * P:k * R_WO + (pt + 1) * P]

        def vo_ap(r, k):
            return ovw[:, KD * R_WO + r * D + k * P:
                       KD * R_WO + r * D + (k + 1) * P]

        # ======== scope A: QKV + attention ========
        with ExitStack() as scA:
            pA = scA.enter_context(tc.tile_pool(name="pA", bufs=1))
            tmpp = scA.enter_context(tc.tile_pool(name="tmpp", bufs=5))
            qkp = scA.enter_context(tc.tile_pool(name="qkp", bufs=3))
            vbp = scA.enter_context(tc.tile_pool(name="vbp", bufs=2))
            probs_pool = scA.enter_context(tc.tile_pool(name="probs", bufs=3))
            small = scA.enter_context(tc.tile_pool(name="small", bufs=2))
            ps_at = scA.enter_context(
                tc.tile_pool(name="ps_at", bufs=1, space="PSUM"))

            p_pack = pA.tile([P, KD, NG * P], bf16, name="p_pack")
            xb = pA.tile([P, KD, M], bf16, name="xb")
            w2 = pA.tile([P, W_TOT], bf16, name="w2")
            tmp_t = {}
            qb_t, kb_t, vb_t = {}, {}, {}

            # -- DMA: few, large transfers; critical stream first --
            pp_r = pp_d.rearrange("(k p) c -> p k c", p=P)
            xb_r = xb_d.rearrange("(k p) m -> p k m", p=P)
            dma(p_pack[:, :, 0:384], pp_r[:, :, 0:384])
            dma(xb[:, :, 0:512], xb_r[:, :, 0:512])
            dma(xb[:, :, 512:1024], xb_r[:, :, 512:1024])
            dma(w2, w2_d[:])
            dma(cpack, cp_d[:])
            dma(vbias_b, vbias_d[0:1, :].to_broadcast((P, 3 * 4 * VW)))
            for c in (1, 2):
                dma(p_pack[:, :, c * 384:(c + 1) * 384],
                    pp_r[:, :, c * 384:(c + 1) * 384])
            dma(xTt, xT_d.rearrange("(k p) m -> p k m", p=P))
            dma(ovw, ov_d[:])
            dma(wpack, wp_d[:])
            dma(wvow, wvo_d[:])
            if aff_d:
                dma(affc, aff_d["a"][:])

            def ph1a_group(t, gq, eng):
                """tmp[g] = p_pack_g.T @ xT; g = 3*gq + t (Q/K/V=0/1/2)"""
                g = 3 * gq + t
                tmp_t[g] = tmpp.tile([P, M], f32, tag="tmp", name=f"tmp{g}")
                ps = big_ps(f"ps1a_{g}")
                for mix, msl in enumerate(MI):
                    for k in range(KD):
                        mm(ps[:, msl], R(p_pack[:, k, g * P:(g + 1) * P]),
                           R(xTt[:, k, msl]), start=(k == 0),
                           stop=(k == KD - 1), skip_group_check=True)
                if eng == 0:
                    nc.vector.tensor_copy(out=tmp_t[g], in_=ps)
                elif eng == 1:
                    nc.scalar.copy(out=tmp_t[g], in_=ps)
                else:
                    nc.gpsimd.tensor_copy(out=tmp_t[g], in_=ps)

            def qk2nd_side(p, side):
                """qb or kb for pair p from one [128,128] stationary."""
                gq = 3 * (p // 2)
                if side == 0:
                    qb_t[p] = qkp.tile([P, M], f32r, tag="qk", bufs=6,
                                       name=f"qb{p}")
                    woff, g, dst, eng, bcol = W_Q, gq, qb_t[p], 0, p
                else:
                    kb_t[p] = qkp.tile([P, M], f32r, tag="qk", bufs=6,
                                       name=f"kb{p}")
                    woff, g, dst, eng, bcol = (W_K, gq + 1, kb_t[p], 2,
                                               NPAIR + p)
                ps = aux_ps(f"ps2nd_{p}_{side}")
                for mix, msl in enumerate(MI):
                    mm(ps[:, msl],
                       w2[:, woff + p * P:woff + (p + 1) * P],
                       tmp_t[g][:, msl],
                       start=True, stop=True, skip_group_check=True)
                nc.vector.tensor_scalar_add(dst[:, MI[0]], ps[:, MI[0]],
                                            qkbias(bcol))
                nc.scalar.activation(out=dst[:, MI[1]], in_=ps[:, MI[1]],
                                     func=AF.Identity, bias=qkbias(bcol))

            def v2nd_j(vg, j):
                """vb[vg] key-chunk j: one matmul of N=260 (4 heads)."""
                g = 3 * vg + 2
                if j == 0:
                    vb_t[vg] = vbp.tile([P, NPT, 4 * VW], bf16, tag="vb",
                                        name=f"vb{vg}")
                ps = (ps_aux.tile([P, 4 * VW], f32, tag="aux",
                                  name=f"psv_{vg}_{j}")
                      if state["use_aux"] else
                      ps_big.tile([P, 4 * VW], f32, tag="big",
                                  name=f"psv_{vg}_{j}"))
                mm(ps, tmp_t[g][:, j * P:(j + 1) * P],
                   w2[:, W_V + vg * 4 * VW:W_V + (vg + 1) * 4 * VW],
                   start=True, stop=True, skip_group_check=True)
                nc.vector.tensor_tensor(
                    out=vb_t[vg][:, j, :], in0=ps,
                    in1=vbias_b[:, vg * 4 * VW:(vg + 1) * 4 * VW],
                    op=OP.add)

            def _emit_xsum():
                """xsum_s = colsum(x)/D + bosum768 (broadcast [128, M])."""
                ps = big_ps("ps_xsum")
                for mix, msl in enumerate(MI):
                    for k in range(KD):
                        mm(ps[:, msl], R(ones_f), R(xTt[:, k, msl]),
                           start=(k == 0), stop=(k == KD - 1),
                           skip_group_check=True)
                nc.vector.tensor_scalar_add(xsum_s, ps, bosum768)

            # filler queue: later chunks' QKV work, pumped a
            # closure at a time inside the attention head loops so the
            # in-order PE stream interleaves it with scores/PV
            filler = []

            def pump(n=1):
                for _ in range(min(n, len(filler))):
                    filler.pop(0)()

            def ph1a_half(t, gq, mix, eng):
                g = 3 * gq + t
                if mix == 0:
                    tmp_t[g] = tmpp.tile([P, M], bf16, tag="tmp",
                                         name=f"tmp{g}")
                    tmp_t[(g, "ps")] = aux_ps(f"ps1a_{g}")
                ps = tmp_t[(g, "ps")]
                msl = MI[mix]
                for k in range(KD):
                    mm(ps[:, msl], p_pack[:, k, g * P:(g + 1) * P],
                       xb[:, k, msl], start=(k == 0),
                       stop=(k == KD - 1), skip_group_check=True)
                if mix == 0:
                    nc.vector.tensor_copy(out=tmp_t[g][:, msl],
                                          in_=ps[:, msl])
                else:
                    nc.scalar.copy(out=tmp_t[g][:, msl], in_=ps[:, msl])

            def ph1a_group(t, gq, eng):
                ph1a_half(t, gq, 0, eng)
                ph1a_half(t, gq, 1, eng)

            def _xsum_half(mix):
                if mix == 0:
                    tmp_t["xs"] = aux_ps("ps_xsum")
                ps = tmp_t["xs"]
                msl = MI[mix]
                for k in range(KD):
                    mm(ps[:, msl], ones_b16, xb[:, k, msl],
                       start=(k == 0), stop=(k == KD - 1),
                       skip_group_check=True)
                nc.vector.tensor_scalar_add(xsum_s[:, msl], ps[:, msl],
                                            bosum768)

            def queue_chunk(gq, extra=()):
                assert not filler
                for c in extra:
                    filler.append(c)
                for t in range(3):
                    for mix in range(2):
                        filler.append(
                            lambda t=t, mix=mix: ph1a_half(t, gq, mix, 2))
                for p_ in (2 * gq, 2 * gq + 1):
                    filler.append(lambda p_=p_: qk2nd_side(p_, 0))
                    filler.append(lambda p_=p_: qk2nd_side(p_, 1))
                for j_ in range(NPT):
                    filler.append(lambda j_=j_: v2nd_j(gq, j_))
                if gq == 1:
                    filler.append(lambda: _xsum_half(0))
                    filler.append(lambda: _xsum_half(1))

            def attention_head(h):
                p, po = h // 2, 64 * (h % 2)
                vg, slot = h // 4, h % 4
                at = ps_at.tile([VW, M], f32, tag="at", name=f"at{h}")
                prs = {}

                def emit_sc(j):
                    sc = big_ps(f"sc{h}_{j}")
                    for mix, msl in enumerate(MI):
                        mm(sc[:, msl],
                           R(kb_t[p][po:po + DH, j * P:(j + 1) * P]),
                           R(qb_t[p][po:po + DH, msl]),
                           start=True, stop=True, skip_group_check=True)
                    prs[j] = probs_pool.tile([P, M], bf16, tag="pr",
                                             name=f"pr{h}_{j}")
                    nc.scalar.activation(out=prs[j], in_=sc, func=AF.Exp,
                                         bias=maskb(j), scale=0.125)

                emit_sc(0)
                for j in range(NPT):
                    if j + 1 < NPT:
                        emit_sc(j + 1)
                    for mix, msl in enumerate(MI):
                        mm(at[:, msl],
                           vb_t[vg][:, j, slot * VW:(slot + 1) * VW],
                           prs[j][:, msl],
                           start=(j == 0), stop=(j == NPT - 1),
                           skip_group_check=True)
                    if j in (1, 3, 5):
                        pump(1)
                # normalize: attn = A/den; 1/den broadcast via K=1 matmul
                # (at evacuated to SBUF first: vector ops may read at most
                # one PSUM operand, and this frees the at slot early)
                rec = small.tile([1, M], f32r, tag="recs", bufs=2,
                                 name=f"rec{h}")
                nc.vector.reciprocal(out=rec, in_=at[DH:VW, :])
                at_s = probs_pool.tile([DH, M], bf16, tag="pr",
                                       name=f"at_s{h}")
                nc.scalar.copy(out=at_s, in_=at[0:DH, :])
                pump(1)
                rb = aux_ps(f"rb{h}")
                for mix, msl in enumerate(MI):
                    mm(rb[0:DH, msl], R(ones_1[0:1, 0:DH]), R(rec[:, msl]),
                       start=True, stop=True, skip_group_check=True)
                nc.vector.tensor_tensor(
                    out=attn_sc[p][po:po + DH, :], in0=at_s,
                    in1=rb[0:DH, :], op=OP.mult)
                pump(1)

            # ---- emission = per-engine execution order ----
            ph1a_group(0, 0, 0)
            ph1a_group(1, 0, 1)
            qk2nd_side(0, 0)
            qk2nd_side(0, 1)
            ph1a_group(2, 0, 2)
            for j_ in range(NPT):
                v2nd_j(0, j_)
            state["use_aux"] = True
            queue_chunk(1, extra=(lambda: qk2nd_side(1, 0),
                                  lambda: qk2nd_side(1, 1)))
            for h in range(4):
                attention_head(h)
            pump(99)
            queue_chunk(2)
            for h in range(4, 8):
                attention_head(h)
            pump(99)
            for h in range(8, 12):
                attention_head(h)

        # ======== outproj + LN1 ========

        # h1 = Uo.T @ attn_sc
        for pt in range(2):
            for mix, msl in enumerate(MI):
                ps = ps_big.tile([P, 512], f32, tag="big",
                                 name=f"ps_h1_{pt}_{mix}")
                for k in range(KD):
                    mm(ps, uo_ap(k, pt), attn_sc[k][:, msl],
                       start=(k == 0), stop=(k == KD - 1),
                       skip_group_check=True)
                if mix == 0:
                    nc.vector.tensor_copy(out=h1b[:, pt, msl], in_=ps)
                else:
                    nc.scalar.copy(out=h1b[:, pt, msl], in_=ps)

        # hoist the sqrt table load off the LN1 critical chain
        nc.scalar.activation(out=half_lnD, in_=half_lnD, func=AF.Sqrt)
        nc.vector.memset(half_lnD, 0.5 * LOG_D)

        # mu1 = colsum(Vo)/D @ h1b + xsum_s
        ps_mu1 = aux_ps("ps_mu1")
        for mix, msl in enumerate(MI):
            for r in range(2):
                mm(ps_mu1[:, msl], wvo_b(r), h1b[:, r, msl],
                   start=(r == 0), stop=(r == 1), skip_group_check=True)
        nc.vector.tensor_tensor(out=mu1_b, in0=ps_mu1, in1=xsum_s, op=OP.add)

        # t[k] = xT[k] - mu1  (in place on xTt; bo folds into the x1c STT)
        for k in range(KD):
            eng = nc.gpsimd if k % 2 == 0 else nc.vector
            eng.tensor_tensor(out=xTt[:, k, :], in0=xTt[:, k, :],
                              in1=mu1_b, op=OP.subtract)

        # vo matmuls + x1c + squares + var1 (var colsums lag vo by 3 so
        # the in-order PE stream never waits on the DVE+ACT sq chain)
        with tc.tile_pool(name="ps_st1", bufs=1, space="PSUM") as ps_st1:
            var1_ps = ps_st1.tile([P, M], f32, tag="stat", name="var1_ps")
            sqs = {}

            def vo_k(k):
                ps = big_ps(f"ps_vo_{k}")
                for mix, msl in enumerate(MI):
                    for r in range(2):
                        mm(ps[:, msl], vo_ap(r, k), h1b[:, r, msl],
                           start=(r == 0), stop=(r == 1),
                           skip_group_check=True)
                nc.vector.scalar_tensor_tensor(
                    out=x1c[k], in0=ps, scalar=boc(k), in1=xTt[:, k, :],
                    op0=OP.add, op1=OP.add)
                sqs[k] = sqp.tile([P, M], f32r, tag="sq", bufs=4,
                                  name=f"sq1_{k}")
                nc.scalar.activation(out=sqs[k], in_=x1c[k], func=AF.Square)

            def var1_k(k):
                for mix, msl in enumerate(MI):
                    mm(var1_ps[:, msl], R(ones_1), R(sqs[k][:, msl]),
                       start=(k == 0), stop=(k == KD - 1),
                       skip_group_check=True)

            LAG = 3
            for k in range(KD):
                vo_k(k)
                if k >= LAG:
                    var1_k(k - LAG)
            for k in range(KD - LAG, KD):
                var1_k(k)

            # a1 = 1/sqrt(var) = sqrt(D / colsum_sq)
            nc.vector.reciprocal(out=lnv, in_=var1_ps)
            nc.scalar.activation(out=a1_b, in_=lnv, func=AF.Sqrt,
                                 scale=float(D))

        aux_ctx.close()   # free the aux PSUM bank for the g2 accumulators

        # ======== FFN ========
        ffw = top.enter_context(tc.tile_pool(name="ffw", bufs=1))
        u1w = ffw.tile([P, KD, R_FF], f32r, name="u1w")
        dma(u1w, u1_d.rearrange("(k p) c -> p k c", p=P))
        v1w = ffw.tile([P, 2, DFF], bf16, name="v1w")
        dma(v1w, v1_d.rearrange("(k p) c -> p k c", p=P))
        u2w = ffw.tile([P, FFT, R_FF], bf16, name="u2w")
        dma(u2w, u2_d.rearrange("(k p) c -> p k c", p=P))
        v2w = ffw.tile([P, 2, D], f32r, name="v2w")
        dma(v2w, v2_d.rearrange("(k p) c -> p k c", p=P))

        ffa = top.enter_context(tc.tile_pool(name="ffa", bufs=1))
        midb = ffa.tile([P, 2, M], bf16, name="midb")
        for pt in range(2):
            for mix, msl in enumerate(MI):
                ps = ps_big.tile([P, 512], f32, tag="big",
                                 name=f"ps_mid_{pt}_{mix}")
                for k in range(KD):
                    mm(ps, R(u1w[:, k, pt * P:(pt + 1) * P]),
                       R(x1c[k][:, msl]), start=(k == 0),
                       stop=(k == KD - 1), skip_group_check=True)
                nc.vector.tensor_tensor(
                    out=midb[:, pt, msl], in0=ps, in1=a1_b[:, msl],
                    op=OP.mult)
                if has_aff1:
                    nc.vector.tensor_scalar_add(
                        midb[:, pt, msl], midb[:, pt, msl],
                        affc[:, 24 + pt:25 + pt])

        # x1full[k] = x1c[k]*a1 (+affine) in place on x1c (LN2 residual)
        for k in range(KD):
            eng = nc.gpsimd if k % 2 == 0 else nc.vector
            eng.tensor_tensor(out=x1c[k], in0=x1c[k], in1=a1_b, op=OP.mult)
            if has_aff1:
                nc.vector.tensor_scalar(
                    out=x1c[k], in0=x1c[k], scalar1=affc[:, k:k + 1],
                    scalar2=affc[:, 6 + k:7 + k], op0=OP.mult, op1=OP.add)

        # dff + gelu + g2 (rotating dffb tiles, 4 parallel g2 accumulators)
        dffp = top.enter_context(tc.tile_pool(name="dffp", bufs=4))
        g2p = top.enter_context(tc.tile_pool(name="g2p", bufs=1))
        g2b = g2p.tile([P, 2, M], f32r, name="g2b")
        with tc.tile_pool(name="ps_g2", bufs=1, space="PSUM") as ps_g2:
            g2ps = [[ps_g2.tile([P, 512], f32, tag=f"g2_{pt}_{mix}",
                                name=f"g2ps_{pt}_{mix}")
                     for mix in range(2)] for pt in range(2)]
            dffts = {}

            def dff_ft(ft):
                ps = big_ps(f"ps_dff_{ft}")
                for mix, msl in enumerate(MI):
                    for r in range(2):
                        mm(ps[:, msl], v1w[:, r, ft * P:(ft + 1) * P],
                           midb[:, r, msl], start=(r == 0), stop=(r == 1),
                           skip_group_check=True)
                dffts[ft] = dffp.tile([P, M], bf16, tag="dffb",
                                      name=f"dffb{ft}")
                nc.scalar.activation(out=dffts[ft], in_=ps, func=AF.Gelu,
                                     bias=b1c(ft))

            def g2_ft(ft):
                for pt in range(2):
                    for mix, msl in enumerate(MI):
                        mm(g2ps[pt][mix], u2w[:, ft, pt * P:(pt + 1) * P],
                           dffts[ft][:, msl], start=(ft == 0),
                           stop=(ft == FFT - 1), skip_group_check=True)

            for ft in range(FFT):
                dff_ft(ft)
                if ft >= 1:
                    g2_ft(ft - 1)
            g2_ft(FFT - 1)
            evac_engs = ((nc.vector.tensor_copy, nc.scalar.copy),
                         (nc.vector.tensor_copy, nc.scalar.copy))
            for pt in range(2):
                for mix, msl in enumerate(MI):
                    evac_engs[pt][mix](out=g2b[:, pt, msl],
                                       in_=g2ps[pt][mix])

        # mu2 = colsum(V2)/D @ g2b + b2sum768 (+ colsum(x1full)/D if affine)
        ps_mu2 = big_ps("ps_mu2")
        for mix, msl in enumerate(MI):
            ops = [(wv2_b(r), g2b[:, r, msl]) for r in range(2)]
            if has_aff1:
                ops += [(ones_f, x1c[k][:, msl]) for k in range(KD)]
            for i, (lhsT, rhs) in enumerate(ops):
                mm(ps_mu2[:, msl], R(lhsT), R(rhs), start=(i == 0),
                   stop=(i == len(ops) - 1), skip_group_check=True)
        nc.vector.tensor_scalar_add(mu2_b, ps_mu2, b2sum768)

        # hoist the gelu->sqrt table swap off the LN2 tail
        nc.scalar.activation(out=lnv[:, 0:1], in_=ones_1[:, 0:1],
                             func=AF.Sqrt)

        # t2[k] = x1full[k] - mu2 (in place on x1c; b2 folds into z2c STT)
        for k in range(KD):
            eng = nc.gpsimd if k % 2 == 0 else nc.vector
            eng.tensor_tensor(out=x1c[k], in0=x1c[k], in1=mu2_b,
                              op=OP.subtract)

        # v2 + z2c + squares + var2 (var colsums lag v2 by 2)
        with tc.tile_pool(name="ps_st2", bufs=1, space="PSUM") as ps_st2:
            var2_ps = ps_st2.tile([P, M], f32, tag="stat", name="var2_ps")
            sq2s = {}

            def v2_k(k):
                ps = big_ps(f"ps_v2_{k}")
                for mix, msl in enumerate(MI):
                    for r in range(2):
                        mm(ps[:, msl], R(v2w[:, r, k * P:(k + 1) * P]),
                           R(g2b[:, r, msl]), start=(r == 0), stop=(r == 1),
                           skip_group_check=True)
                nc.vector.scalar_tensor_tensor(
                    out=xTt[:, k, :], in0=ps, scalar=b2c(k),
                    in1=x1c[k], op0=OP.add, op1=OP.add)
                sq2s[k] = sqp.tile([P, M], f32r, tag="sq", bufs=4,
                                   name=f"sq2_{k}")
                nc.scalar.activation(out=sq2s[k], in_=xTt[:, k, :],
                                     func=AF.Square)

            def var2_k(k):
                for mix, msl in enumerate(MI):
                    mm(var2_ps[:, msl], R(ones_1), R(sq2s[k][:, msl]),
                       start=(k == 0), stop=(k == KD - 1),
                       skip_group_check=True)

            LAG2 = 2
            for k in range(KD):
                v2_k(k)
                if k >= LAG2:
                    var2_k(k - LAG2)
            for k in range(KD - LAG2, KD):
                var2_k(k)

            nc.vector.reciprocal(out=lnv, in_=var2_ps)
            nc.scalar.activation(out=a2_b, in_=lnv, func=AF.Sqrt,
                                 scale=float(D))

        for k in range(KD):
            eng = nc.gpsimd if k in (0, 2) else nc.vector
            if has_aff2:
                eng.tensor_tensor(out=xTt[:, k, :], in0=xTt[:, k, :],
                                  in1=a2_b, op=OP.mult)
                nc.vector.tensor_scalar(
                    out=attn_sc[k], in0=xTt[:, k, :],
                    scalar1=affc[:, 12 + k:13 + k],
                    scalar2=affc[:, 18 + k:19 + k], op0=OP.mult, op1=OP.add)
            else:
                eng.tensor_tensor(out=attn_sc[k], in0=xTt[:, k, :],
                                  in1=a2_b, op=OP.mult)
            dma(out_d[k * P:(k + 1) * P, :], attn_sc[k])

    nc.compile()
    return nc


def _prep_inputs(x, mask, Pq, Vq, bq, Pk, Vk, bk, Pv, Vv, bv,
                 Uo, Vo, bo_attn, U1, V1, b1, U2, V2, b2,
                 ln1_g, ln1_b, ln2_g, ln2_b):
    """Host-side packing: per-core in_maps for the SPMD kernel."""
    has_aff1 = not (np.all(ln1_g == 1.0) and np.all(ln1_b == 0.0))
    has_aff2 = not (np.all(ln2_g == 1.0) and np.all(ln2_b == 0.0))

    # p_pack: group order [Q0 K0 V0 Q1 K1 V1 Q2 K2 V2]; 4 heads x 32/group
    p_pack = np.zeros((D, NG * P), np.float32)
    for t, Pw in enumerate((Pq, Pk, Pv)):
        for h in range(H):
            g = 3 * (h // 4) + t
            c0 = g * P + 32 * (h % 4)
            p_pack[:, c0:c0 + 32] = Pw[h]

    # w2pack: Q pairs | K pairs | V groups
    w2 = np.zeros((P, W_TOT), np.float32)
    for tt, Vw in ((0, Vq), (1, Vk)):
        for p in range(NPAIR):
            for s in range(2):
                h = 2 * p + s
                r0 = 32 * (h % 4)
                c0 = tt * NPAIR * P + p * P + 64 * s
                w2[r0:r0 + 32, c0:c0 + DH] = Vw[h]
    vbias = np.zeros((1, 3 * 4 * VW), np.float32)
    for h in range(H):
        vg, i = h // 4, h % 4
        w2[32 * i:32 * i + 32,
           W_V + vg * 4 * VW + VW * i:W_V + vg * 4 * VW + VW * i + DH] = Vv[h]
        vbias[0, vg * 4 * VW + VW * i:vg * 4 * VW + VW * i + DH] = \
            bv[0, h, 0, :]
        vbias[0, vg * 4 * VW + VW * i + DH] = 1.0

    # cpack [128, 56] fp32 (maskb filled per-core below)
    cpack = np.zeros((P, C_TOT), np.float32)
    cpack[:, C_BO:C_BO + KD] = np.asarray(bo_attn, np.float32).reshape(KD, P).T
    cpack[:, C_B2:C_B2 + KD] = np.asarray(b2, np.float32).reshape(KD, P).T
    cpack[:, C_B1:C_B1 + FFT] = np.asarray(b1, np.float32).reshape(FFT, P).T
    for p in range(NPAIR):
        cpack[0:DH, C_QKB + p] = bq[0, 2 * p, 0, :]
        cpack[DH:P, C_QKB + p] = bq[0, 2 * p + 1, 0, :]
        cpack[0:DH, C_QKB + NPAIR + p] = bk[0, 2 * p, 0, :]
        cpack[DH:P, C_QKB + NPAIR + p] = bk[0, 2 * p + 1, 0, :]

    def bcast_colsum(Vw):
        w = np.asarray(Vw, np.float32).sum(axis=1) / D   # [256]
        out = np.zeros((P, 2 * P), np.float32)
        for r in range(2):
            out[:, r * P:(r + 1) * P] = w[r * P:(r + 1) * P][:, None]
        return out

    wpack = np.ascontiguousarray(bcast_colsum(V2), np.float32)
    wvopack = bcast_colsum(Vo).astype(BF16)

    u1_eff = np.asarray(U1, np.float32)
    if has_aff1:
        u1_eff = np.asarray(ln1_g, np.float32)[:, None] * u1_eff

    def rpack(a, kd):
        a = np.asarray(a, np.float32)
        return a.reshape(kd, P, -1).transpose(1, 0, 2).reshape(P, -1)

    ovpack = np.concatenate(
        [rpack(Uo, KD), rpack(Vo, 2)], axis=1).astype(BF16)

    bosum768 = float(np.asarray(bo_attn, np.float32).sum() / D)
    b2sum768 = float(np.asarray(b2, np.float32).sum() / D)

    common = {
        "p_pack": p_pack.astype(BF16), "w2pack": w2.astype(BF16),
        "vbias": vbias, "wpack": wpack,
        "wvopack": wvopack,
        "ovpack": ovpack,
        "u1": np.ascontiguousarray(u1_eff, np.float32),
        "v1": np.asarray(V1, np.float32).astype(BF16),
        "u2": np.asarray(U2, np.float32).astype(BF16),
        "v2": np.ascontiguousarray(V2, np.float32),
    }
    if has_aff1 or has_aff2:
        affp = np.zeros((P, 26), np.float32)
        affp[:, 0:KD] = np.asarray(ln1_g, np.float32).reshape(KD, P).T
        affp[:, 6:6 + KD] = np.asarray(ln1_b, np.float32).reshape(KD, P).T
        affp[:, 12:12 + KD] = np.asarray(ln2_g, np.float32).reshape(KD, P).T
        affp[:, 18:18 + KD] = np.asarray(ln2_b, np.float32).reshape(KD, P).T
        mc1 = np.asarray(U1, np.float32).T @ np.asarray(ln1_b, np.float32)
        affp[:, 24:26] = mc1.reshape(2, P).T
        common["affpack"] = affp

    in_maps = []
    for b_i in range(B):
        m = dict(common)
        xt = np.ascontiguousarray(x[b_i].T, np.float32)
        m["xT"] = xt
        m["xb"] = xt.astype(BF16)
        cpk = cpack.copy()
        mb = np.where(mask[b_i] > 0, 0.0, -1e9).astype(np.float32)
        cpk[:, C_MASK:C_MASK + NPT] = mb.reshape(NPT, P).T
        m["cpack"] = cpk
        in_maps.append(m)
    return in_maps, has_aff1, has_aff2, bosum768, b2sum768


def build_program_for_inputs(**inputs):
    inputs = {k: np.asarray(v) for k, v in inputs.items()}
    in_maps, has_aff1, has_aff2, bosum768, b2sum768 = _prep_inputs(**inputs)
    key = (has_aff1, has_aff2, round(bosum768, 12), round(b2sum768, 12))
    if key not in _prog_cache:
        _prog_cache[key] = _build_program(has_aff1, has_aff2,
                                          bosum768, b2sum768)
    return _prog_cache[key], in_maps


def kernel(**inputs):
    global last_results
    nc, in_maps = build_program_for_inputs(**inputs)
    from concourse.bass_utils import run_bass_kernel_spmd
    res = run_bass_kernel_spmd(nc, in_maps, list(range(N_CORES)))
    last_results = res
    out = np.stack([np.asarray(res.results[b]["outT"], np.float32).T
                    for b in range(B)])
    return np.ascontiguousarray(out, np.float32)


# revision 5
# speedup vs baseline: 1.0294x; 1.0294x over previous
"""Trainium2 Bass kernel for nn_BertSVDBlock (B=8, M=1024, D=768, H=12).

Sharding: pure data-parallel over batch B - core b computes batch element b.

v3: everything fp32/float32r. TRN2 fp32r matmuls at N>=256 run at full
rate AND are self-loading (no separate LDWEIGHTS instruction), which
removes ~860 x 71ns of PE-sequencer issue overhead vs the bf16 design,
and improves precision. Other key points (all transposed layout [d|r, m]):
  - QKV first factors packed 4 heads/group, group order Q0 K0 V0 Q1 ...
    computed straight from xT (no bf16 x copy).
  - Q/K biases folded at PSUM evacuation (per-partition tensor_scalar);
    V bias + denominator-ones column via one broadcast TT add.
  - Q/K second factors head-pair packed; V second factors 4-heads/matmul.
  - Softmax denominators: DVE reciprocal, partition-broadcast via K=1
    fp32r ones-matmul into PSUM (no DRAM bounce).
  - LayerNorms: means folded out algebraically (colsum(x)/D early
    matmuls; host-precomputed colsum(Vo)/D, colsum(V2)/D broadcast
    stationaries; colsum(x1)=0). Stats as [128,M] broadcast tiles from
    ones-matmuls; variance from centered squares; rsqrt as
    exp(-0.5*ln(colsum_sq)+0.5*lnD) on ACT. LN1's per-column scale a1
    folded THROUGH the U1 matmul.
  - ~20 large DMAs total (the sim serializes DMA issue).
"""

import os
import sys

import numpy as np

import ml_dtypes

BF16 = ml_dtypes.bfloat16

for _p in ("/opt/trn_rl_repo", "/root/.axon_site/_ro/trn_rl_repo"):
    if os.path.isdir(_p) and _p not in sys.path:
        sys.path.append(_p)

B, M, D, H, DH = 8, 1024, 768, 12, 64
R_ATTN, R_FF, R_WO, DFF = 32, 256, 256, 3072
LN_EPS = 1e-12
N_CORES = 8
P = 128
KD = D // P            # 6 d-chunks
NPT = M // P           # 8 key chunks
FFT = DFF // P         # 24 dff chunks
NG = 9                 # 9 col groups in p_pack, order [Q K V] x 3
NPAIR = 6              # head pairs
VW = DH + 1            # 65: V columns + denominator-ones column
LOG_D = float(np.log(D))

# cpack column layout (per-partition fp32 consts)
C_MASK, C_BO, C_B2, C_B1, C_QKB = 0, 8, 14, 20, 44
C_TOT = 56
# w2pack column layout
W_Q, W_K, W_V = 0, NPAIR * P, 2 * NPAIR * P
W_TOT = 2 * NPAIR * P + 3 * 4 * VW     # 2316

_prog_cache: dict = {}
last_results = None


def _build_program(has_aff1: bool, has_aff2: bool,
                   bosum768: float, b2sum768: float):
    from contextlib import ExitStack

    import concourse.tile as tile
    from concourse import bacc
    from concourse import mybir

    f32 = mybir.dt.float32
    f32r = mybir.dt.float32r
    bf16 = mybir.dt.bfloat16
    AF = mybir.ActivationFunctionType
    OP = mybir.AluOpType

    nc = bacc.Bacc("TRN2", target_bir_lowering=False)

    def R(ap):
        return ap.bitcast(f32r)

    # ---- I/O (all fp32) ----
    xT_d = nc.dram_tensor("xT", [D, M], f32, kind="ExternalInput")
    pp_d = nc.dram_tensor("p_pack", [D, NG * P], bf16,
                          kind="ExternalInput")
    xb_d = nc.dram_tensor("xb", [D, M], bf16, kind="ExternalInput")
    w2_d = nc.dram_tensor("w2pack", [P, W_TOT], bf16,
                          kind="ExternalInput")
    cp_d = nc.dram_tensor("cpack", [P, C_TOT], f32, kind="ExternalInput")
    vbias_d = nc.dram_tensor("vbias", [1, 3 * 4 * VW], f32,
                             kind="ExternalInput")
    wp_d = nc.dram_tensor("wpack", [P, 2 * P], f32r,
                          kind="ExternalInput")
    wvo_d = nc.dram_tensor("wvopack", [P, 2 * P], bf16,
                           kind="ExternalInput")
    ov_d = nc.dram_tensor("ovpack", [P, KD * R_WO + 2 * D], bf16,
                          kind="ExternalInput")
    u1_d = nc.dram_tensor("u1", [D, R_FF], f32r, kind="ExternalInput")
    v1_d = nc.dram_tensor("v1", [R_FF, DFF], bf16, kind="ExternalInput")
    u2_d = nc.dram_tensor("u2", [DFF, R_FF], bf16, kind="ExternalInput")
    v2_d = nc.dram_tensor("v2", [R_FF, D], f32r, kind="ExternalInput")
    aff_d = {}
    if has_aff1 or has_aff2:
        aff_d["a"] = nc.dram_tensor("affpack", [P, 26], f32,
                                    kind="ExternalInput")
    out_d = nc.dram_tensor("outT", [D, M], bf16, kind="ExternalOutput")

    MI = (slice(0, 512), slice(512, 1024))

    with ExitStack() as top:
        # float32r outputs are fp32-width; the low-precision guard is about
        # 16-bit accumulation, which none of these ops do
        top.enter_context(nc.allow_low_precision(
            reason="float32r tiles are fp32-width"))
        tc = top.enter_context(tile.TileContext(nc))
        dma = nc.sync.dma_start
        mm = nc.tensor.matmul

        consts = top.enter_context(tc.tile_pool(name="consts", bufs=1))

        ones_f = consts.tile([P, P], f32r, name="ones_f")   # 1/D (bf16 MMs)
        nc.vector.memset(ones_f.bitcast(f32), 1.0 / D)
        ones_b16 = consts.tile([P, P], bf16, name="ones_b16")
        nc.vector.memset(ones_b16, 1.0 / D)
        ones_1 = consts.tile([P, P], f32r, name="ones_1")   # 1.0
        nc.vector.memset(ones_1.bitcast(f32), 1.0)
        half_lnD = consts.tile([P, 1], f32, name="half_lnD")
        nc.vector.memset(half_lnD, 0.5 * LOG_D)
        cpack = consts.tile([P, C_TOT], f32, name="cpack")
        vbias_b = consts.tile([P, 3 * 4 * VW], f32, name="vbias_b")
        wpack = consts.tile([P, 2 * P], f32r, name="wpack")
        wvow = consts.tile([P, 2 * P], bf16, name="wvow")
        affc = consts.tile([P, 26], f32, name="affc") if aff_d else None

        def cp(col, n=1):
            return cpack[:, col:col + n]

        maskb = lambda j: cp(C_MASK + j)          # noqa: E731
        boc = lambda k: cp(C_BO + k)              # noqa: E731
        b2c = lambda k: cp(C_B2 + k)              # noqa: E731
        b1c = lambda t: cp(C_B1 + t)              # noqa: E731
        qkbias = lambda i: cp(C_QKB + i)          # noqa: E731
        wvo_b = lambda r: wvow[:, r * P:(r + 1) * P]       # noqa: E731
        wv2_b = lambda r: wpack[:, r * P:(r + 1) * P]      # noqa: E731

        # persistent activation tiles
        keep = top.enter_context(tc.tile_pool(name="keep", bufs=1))
        xTt = keep.tile([P, KD, M], f32, name="xTt")
        xsum_s = keep.tile([P, M], f32, name="xsum_s")

        x1p = top.enter_context(tc.tile_pool(name="x1p", bufs=1))
        x1c = [x1p.tile([P, M], f32r, name=f"x1c{k}")
               for k in range(KD)]
        sqp = top.enter_context(tc.tile_pool(name="sqp", bufs=4))
        stats = top.enter_context(tc.tile_pool(name="stats", bufs=1))
        mu1_b = stats.tile([P, M], f32, name="mu1_b")
        a1_b = stats.tile([P, M], f32, name="a1_b")
        mu2_b = stats.tile([P, M], f32, name="mu2_b")
        a2_b = stats.tile([P, M], f32, name="a2_b")
        lnv = stats.tile([P, M], f32, name="lnv", tag="lnv", bufs=1)

        # PSUM: 2-slot pool reserved for score tiles (and later-phase
        # rotating psums); 1-slot aux pool for filler/broadcast psums
        ps_big = top.enter_context(
            tc.tile_pool(name="ps_big", bufs=2, space="PSUM"))

        def big_ps(name):
            return ps_big.tile([P, M], f32, tag="big", name=name)

        aux_ctx = top.enter_context(ExitStack())
        ps_aux = aux_ctx.enter_context(
            tc.tile_pool(name="ps_aux", bufs=1, space="PSUM"))
        state = {"use_aux": False}

        def aux_ps(name):
            if state["use_aux"]:
                return ps_aux.tile([P, M], f32, tag="aux", name=name)
            return big_ps(name)

        # outproj tensors (bf16: saves SBUF; ~52 extra LDWEIGHTS only)
        pB = top.enter_context(tc.tile_pool(name="pB", bufs=1))
        attn_sc = [pB.tile([P, M], bf16, name=f"attn_sc{k}")
                   for k in range(KD)]
        h1b = pB.tile([P, 2, M], bf16, name="h1b")
        ovw = pB.tile([P, KD * R_WO + 2 * D], bf16, name="ovw")

        def uo_ap(k, pt):
            return ovw[:, k * R_WO + pt * P:k * R_WO + (pt + 1) * P]

        def vo_ap(r, k):
            return ovw[:, KD * R_WO + r * D + k * P:
                       KD * R_WO + r * D + (k + 1) * P]

        # ======== scope A: QKV + attention ========
        with ExitStack() as scA:
            pA = scA.enter_context(tc.tile_pool(name="pA", bufs=1))
            tmpp = scA.enter_context(tc.tile_pool(name="tmpp", bufs=5))
            qkp = scA.enter_context(tc.tile_pool(name="qkp", bufs=3))
            vbp = scA.enter_context(tc.tile_pool(name="vbp", bufs=2))
            probs_pool = scA.enter_context(tc.tile_pool(name="probs", bufs=3))
            small = scA.enter_context(tc.tile_pool(name="small", bufs=2))
            ps_at = scA.enter_context(
                tc.tile_pool(name="ps_at", bufs=1, space="PSUM"))

            p_pack = pA.tile([P, KD, NG * P], bf16, name="p_pack")
            xb = pA.tile([P, KD, M], bf16, name="xb")
            w2 = pA.tile([P, W_TOT], bf16, name="w2")
            tmp_t = {}
            qb_t, kb_t, vb_t = {}, {}, {}

            # -- DMA: few, large transfers; critical stream first --
            pp_r = pp_d.rearrange("(k p) c -> p k c", p=P)
            xb_r = xb_d.rearrange("(k p) m -> p k m", p=P)
            dma(p_pack[:, :, 0:384], pp_r[:, :, 0:384])
            dma(xb[:, :, 0:512], xb_r[:, :, 0:512])
            dma(xb[:, :, 512:1024], xb_r[:, :, 512:1024])
            dma(w2, w2_d[:])
            dma(cpack, cp_d[:])
            dma(vbias_b, vbias_d[0:1, :].to_broadcast((P, 3 * 4 * VW)))
            for c in (1, 2):
                dma(p_pack[:, :, c * 384:(c + 1) * 384],
                    pp_r[:, :, c * 384:(c + 1) * 384])
            dma(xTt, xT_d.rearrange("(k p) m -> p k m", p=P))
            dma(ovw, ov_d[:])
            dma(wpack, wp_d[:])
            dma(wvow, wvo_d[:])
            if aff_d:
                dma(affc, aff_d["a"][:])

            def ph1a_group(t, gq, eng):
                """tmp[g] = p_pack_g.T @ xT; g = 3*gq + t (Q/K/V=0/1/2)"""
                g = 3 * gq + t
                tmp_t[g] = tmpp.tile([P, M], f32, tag="tmp", name=f"tmp{g}")
                ps = big_ps(f"ps1a_{g}")
                for mix, msl in enumerate(MI):
                    for k in range(KD):
                        mm(ps[:, msl], R(p_pack[:, k, g * P:(g + 1) * P]),
                           R(xTt[:, k, msl]), start=(k == 0),
                           stop=(k == KD - 1), skip_group_check=True)
                if eng == 0:
                    nc.vector.tensor_copy(out=tmp_t[g], in_=ps)
                elif eng == 1:
                    nc.scalar.copy(out=tmp_t[g], in_=ps)
                else:
                    nc.gpsimd.tensor_copy(out=tmp_t[g], in_=ps)

            def qk2nd_side(p, side):
                """qb or kb for pair p from one [128,128] stationary."""
                gq = 3 * (p // 2)
                if side == 0:
                    qb_t[p] = qkp.tile([P, M], f32r, tag="qk", bufs=6,
                                       name=f"qb{p}")
                    woff, g, dst, eng, bcol = W_Q, gq, qb_t[p], 0, p
                else:
                    kb_t[p] = qkp.tile([P, M], f32r, tag="qk", bufs=6,
                                       name=f"kb{p}")
                    woff, g, dst, eng, bcol = (W_K, gq + 1, kb_t[p], 2,
                                               NPAIR + p)
                ps = aux_ps(f"ps2nd_{p}_{side}")
                for mix, msl in enumerate(MI):
                    mm(ps[:, msl],
                       w2[:, woff + p * P:woff + (p + 1) * P],
                       tmp_t[g][:, msl],
                       start=True, stop=True, skip_group_check=True)
                nc.vector.tensor_scalar_add(dst, ps, qkbias(bcol))

            def v2nd_j(vg, j):
                """vb[vg] key-chunk j: one matmul of N=260 (4 heads)."""
                g = 3 * vg + 2
                if j == 0:
                    vb_t[vg] = vbp.tile([P, NPT, 4 * VW], bf16, tag="vb",
                                        name=f"vb{vg}")
                ps = (ps_aux.tile([P, 4 * VW], f32, tag="aux",
                                  name=f"psv_{vg}_{j}")
                      if state["use_aux"] else
                      ps_big.tile([P, 4 * VW], f32, tag="big",
                                  name=f"psv_{vg}_{j}"))
                mm(ps, tmp_t[g][:, j * P:(j + 1) * P],
                   w2[:, W_V + vg * 4 * VW:W_V + (vg + 1) * 4 * VW],
                   start=True, stop=True, skip_group_check=True)
                nc.vector.tensor_tensor(
                    out=vb_t[vg][:, j, :], in0=ps,
                    in1=vbias_b[:, vg * 4 * VW:(vg + 1) * 4 * VW],
                    op=OP.add)

            def _emit_xsum():
                """xsum_s = colsum(x)/D + bosum768 (broadcast [128, M])."""
                ps = big_ps("ps_xsum")
                for mix, msl in enumerate(MI):
                    for k in range(KD):
                        mm(ps[:, msl], R(ones_f), R(xTt[:, k, msl]),
                           start=(k == 0), stop=(k == KD - 1),
                           skip_group_check=True)
                nc.vector.tensor_scalar_add(xsum_s, ps, bosum768)

            # filler queue: later chunks' QKV work, pumped a
            # closure at a time inside the attention head loops so the
            # in-order PE stream interleaves it with scores/PV
            filler = []

            def pump(n=1):
                for _ in range(min(n, len(filler))):
                    filler.pop(0)()

            def ph1a_half(t, gq, mix, eng):
                g = 3 * gq + t
                if mix == 0:
                    tmp_t[g] = tmpp.tile([P, M], bf16, tag="tmp",
                                         name=f"tmp{g}")
                    tmp_t[(g, "ps")] = aux_ps(f"ps1a_{g}")
                ps = tmp_t[(g, "ps")]
                msl = MI[mix]
                for k in range(KD):
                    mm(ps[:, msl], p_pack[:, k, g * P:(g + 1) * P],
                       xb[:, k, msl], start=(k == 0),
                       stop=(k == KD - 1), skip_group_check=True)
                nc.vector.tensor_copy(out=tmp_t[g][:, msl], in_=ps[:, msl])

            def ph1a_group(t, gq, eng):
                ph1a_half(t, gq, 0, eng)
                ph1a_half(t, gq, 1, eng)

            def _xsum_half(mix):
                if mix == 0:
                    tmp_t["xs"] = aux_ps("ps_xsum")
                ps = tmp_t["xs"]
                msl = MI[mix]
                for k in range(KD):
                    mm(ps[:, msl], ones_b16, xb[:, k, msl],
                       start=(k == 0), stop=(k == KD - 1),
                       skip_group_check=True)
                nc.vector.tensor_scalar_add(xsum_s[:, msl], ps[:, msl],
                                            bosum768)

            def queue_chunk(gq, extra=()):
                assert not filler
                for c in extra:
                    filler.append(c)
                for t in range(3):
                    for mix in range(2):
                        filler.append(
                            lambda t=t, mix=mix: ph1a_half(t, gq, mix, 2))
                for p_ in (2 * gq, 2 * gq + 1):
                    filler.append(lambda p_=p_: qk2nd_side(p_, 0))
                    filler.append(lambda p_=p_: qk2nd_side(p_, 1))
                for j_ in range(NPT):
                    filler.append(lambda j_=j_: v2nd_j(gq, j_))
                if gq == 1:
                    filler.append(lambda: _xsum_half(0))
                    filler.append(lambda: _xsum_half(1))

            def attention_head(h):
                p, po = h // 2, 64 * (h % 2)
                vg, slot = h // 4, h % 4
                at = ps_at.tile([VW, M], f32, tag="at", name=f"at{h}")
                prs = {}

                def emit_sc(j):
                    sc = big_ps(f"sc{h}_{j}")
                    for mix, msl in enumerate(MI):
                        mm(sc[:, msl],
                           R(kb_t[p][po:po + DH, j * P:(j + 1) * P]),
                           R(qb_t[p][po:po + DH, msl]),
                           start=True, stop=True, skip_group_check=True)
                    prs[j] = probs_pool.tile([P, M], bf16, tag="pr",
                                             name=f"pr{h}_{j}")
                    nc.scalar.activation(out=prs[j], in_=sc, func=AF.Exp,
                                         bias=maskb(j), scale=0.125)

                emit_sc(0)
                for j in range(NPT):
                    if j + 1 < NPT:
                        emit_sc(j + 1)
                    for mix, msl in enumerate(MI):
                        mm(at[:, msl],
                           vb_t[vg][:, j, slot * VW:(slot + 1) * VW],
                           prs[j][:, msl],
                           start=(j == 0), stop=(j == NPT - 1),
                           skip_group_check=True)
                    if j in (1, 3, 5):
                        pump(1)
                # normalize: attn = A/den; 1/den broadcast via K=1 matmul
                # (at evacuated to SBUF first: vector ops may read at most
                # one PSUM operand, and this frees the at slot early)
                rec = small.tile([1, M], f32r, tag="recs", bufs=2,
                                 name=f"rec{h}")
                nc.vector.reciprocal(out=rec, in_=at[DH:VW, :])
                at_s = probs_pool.tile([DH, M], bf16, tag="pr",
                                       name=f"at_s{h}")
                if h >= 11:
                    nc.scalar.copy(out=at_s, in_=at[0:DH, :])
                else:
                    nc.vector.tensor_copy(out=at_s, in_=at[0:DH, :])
                pump(1)
                rb = aux_ps(f"rb{h}")
                for mix, msl in enumerate(MI):
                    mm(rb[0:DH, msl], R(ones_1[0:1, 0:DH]), R(rec[:, msl]),
                       start=True, stop=True, skip_group_check=True)
                nc.vector.tensor_tensor(
                    out=attn_sc[p][po:po + DH, :], in0=at_s,
                    in1=rb[0:DH, :], op=OP.mult)
                pump(1)

            # ---- emission = per-engine execution order ----
            ph1a_group(0, 0, 0)
            ph1a_group(1, 0, 1)
            qk2nd_side(0, 0)
            qk2nd_side(0, 1)
            ph1a_group(2, 0, 2)
            for j_ in range(NPT):
                v2nd_j(0, j_)
            state["use_aux"] = True
            queue_chunk(1, extra=(lambda: qk2nd_side(1, 0),
                                  lambda: qk2nd_side(1, 1)))
            for h in range(4):
                attention_head(h)
            pump(99)
            queue_chunk(2)
            for h in range(4, 8):
                attention_head(h)
            pump(99)
            for h in range(8, 12):
                attention_head(h)

        # ======== outproj + LN1 ========

        # h1 = Uo.T @ attn_sc
        for pt in range(2):
            for mix, msl in enumerate(MI):
                ps = ps_big.tile([P, 512], f32, tag="big",
                                 name=f"ps_h1_{pt}_{mix}")
                for k in range(KD):
                    mm(ps, uo_ap(k, pt), attn_sc[k][:, msl],
                       start=(k == 0), stop=(k == KD - 1),
                       skip_group_check=True)
                if mix == 0:
                    nc.vector.tensor_copy(out=h1b[:, pt, msl], in_=ps)
                else:
                    nc.scalar.copy(out=h1b[:, pt, msl], in_=ps)

        # hoist the sqrt table load off the LN1 critical chain
        nc.scalar.activation(out=half_lnD, in_=half_lnD, func=AF.Sqrt)
        nc.vector.memset(half_lnD, 0.5 * LOG_D)

        # mu1 = colsum(Vo)/D @ h1b + xsum_s
        ps_mu1 = aux_ps("ps_mu1")
        for mix, msl in enumerate(MI):
            for r in range(2):
                mm(ps_mu1[:, msl], wvo_b(r), h1b[:, r, msl],
                   start=(r == 0), stop=(r == 1), skip_group_check=True)
        nc.vector.tensor_tensor(out=mu1_b, in0=ps_mu1, in1=xsum_s, op=OP.add)

        # t[k] = xT[k] - mu1  (in place on xTt; bo folds into the x1c STT)
        for k in range(KD):
            eng = nc.gpsimd if k % 2 == 0 else nc.vector
            eng.tensor_tensor(out=xTt[:, k, :], in0=xTt[:, k, :],
                              in1=mu1_b, op=OP.subtract)

        # vo matmuls + x1c + squares + var1 (var colsums lag vo by 3 so
        # the in-order PE stream never waits on the DVE+ACT sq chain)
        with tc.tile_pool(name="ps_st1", bufs=1, space="PSUM") as ps_st1:
            var1_ps = ps_st1.tile([P, M], f32, tag="stat", name="var1_ps")
            sqs = {}

            def vo_k(k):
                ps = big_ps(f"ps_vo_{k}")
                for mix, msl in enumerate(MI):
                    for r in range(2):
                        mm(ps[:, msl], vo_ap(r, k), h1b[:, r, msl],
                           start=(r == 0), stop=(r == 1),
                           skip_group_check=True)
                nc.vector.scalar_tensor_tensor(
                    out=x1c[k], in0=ps, scalar=boc(k), in1=xTt[:, k, :],
                    op0=OP.add, op1=OP.add)
                sqs[k] = sqp.tile([P, M], f32r, tag="sq", bufs=4,
                                  name=f"sq1_{k}")
                nc.scalar.activation(out=sqs[k], in_=x1c[k], func=AF.Square)

            def var1_k(k):
                for mix, msl in enumerate(MI):
                    mm(var1_ps[:, msl], R(ones_1), R(sqs[k][:, msl]),
                       start=(k == 0), stop=(k == KD - 1),
                       skip_group_check=True)

            LAG = 3
            for k in range(KD):
                vo_k(k)
                if k >= LAG:
                    var1_k(k - LAG)
            for k in range(KD - LAG, KD):
                var1_k(k)

            # a1 = 1/sqrt(var) = sqrt(D / colsum_sq)
            nc.vector.reciprocal(out=lnv, in_=var1_ps)
            nc.scalar.activation(out=a1_b, in_=lnv, func=AF.Sqrt,
                                 scale=float(D))

        aux_ctx.close()   # free the aux PSUM bank for the g2 accumulators

        # ======== FFN ========
        ffw = top.enter_context(tc.tile_pool(name="ffw", bufs=1))
        u1w = ffw.tile([P, KD, R_FF], f32r, name="u1w")
        dma(u1w, u1_d.rearrange("(k p) c -> p k c", p=P))
        v1w = ffw.tile([P, 2, DFF], bf16, name="v1w")
        dma(v1w, v1_d.rearrange("(k p) c -> p k c", p=P))
        u2w = ffw.tile([P, FFT, R_FF], bf16, name="u2w")
        dma(u2w, u2_d.rearrange("(k p) c -> p k c", p=P))
        v2w = ffw.tile([P, 2, D], f32r, name="v2w")
        dma(v2w, v2_d.rearrange("(k p) c -> p k c", p=P))

        ffa = top.enter_context(tc.tile_pool(name="ffa", bufs=1))
        midb = ffa.tile([P, 2, M], bf16, name="midb")
        for pt in range(2):
            for mix, msl in enumerate(MI):
                ps = ps_big.tile([P, 512], f32, tag="big",
                                 name=f"ps_mid_{pt}_{mix}")
                for k in range(KD):
                    mm(ps, R(u1w[:, k, pt * P:(pt + 1) * P]),
                       R(x1c[k][:, msl]), start=(k == 0),
                       stop=(k == KD - 1), skip_group_check=True)
                nc.vector.tensor_tensor(
                    out=midb[:, pt, msl], in0=ps, in1=a1_b[:, msl],
                    op=OP.mult)
                if has_aff1:
                    nc.vector.tensor_scalar_add(
                        midb[:, pt, msl], midb[:, pt, msl],
                        affc[:, 24 + pt:25 + pt])

        # x1full[k] = x1c[k]*a1 (+affine) in place on x1c (LN2 residual)
        for k in range(KD):
            eng = nc.gpsimd if k % 2 == 0 else nc.vector
            eng.tensor_tensor(out=x1c[k], in0=x1c[k], in1=a1_b, op=OP.mult)
            if has_aff1:
                nc.vector.tensor_scalar(
                    out=x1c[k], in0=x1c[k], scalar1=affc[:, k:k + 1],
                    scalar2=affc[:, 6 + k:7 + k], op0=OP.mult, op1=OP.add)

        # dff + gelu + g2 (rotating dffb tiles, 4 parallel g2 accumulators)
        dffp = top.enter_context(tc.tile_pool(name="dffp", bufs=4))
        g2p = top.enter_context(tc.tile_pool(name="g2p", bufs=1))
        g2b = g2p.tile([P, 2, M], f32r, name="g2b")
        with tc.tile_pool(name="ps_g2", bufs=1, space="PSUM") as ps_g2:
            g2ps = [[ps_g2.tile([P, 512], f32, tag=f"g2_{pt}_{mix}",
                                name=f"g2ps_{pt}_{mix}")
                     for mix in range(2)] for pt in range(2)]
            dffts = {}

            def dff_ft(ft):
                ps = big_ps(f"ps_dff_{ft}")
                for mix, msl in enumerate(MI):
                    for r in range(2):
                        mm(ps[:, msl], v1w[:, r, ft * P:(ft + 1) * P],
                           midb[:, r, msl], start=(r == 0), stop=(r == 1),
                           skip_group_check=True)
                dffts[ft] = dffp.tile([P, M], bf16, tag="dffb",
                                      name=f"dffb{ft}")
                nc.scalar.activation(out=dffts[ft], in_=ps, func=AF.Gelu,
                                     bias=b1c(ft))

            def g2_ft(ft):
                for pt in range(2):
                    for mix, msl in enumerate(MI):
                        mm(g2ps[pt][mix], u2w[:, ft, pt * P:(pt + 1) * P],
                           dffts[ft][:, msl], start=(ft == 0),
                           stop=(ft == FFT - 1), skip_group_check=True)

            for ft in range(FFT):
                dff_ft(ft)
                if ft >= 1:
                    g2_ft(ft - 1)
            g2_ft(FFT - 1)
            evac_engs = ((nc.vector.tensor_copy, nc.vector.tensor_copy),
                         (nc.scalar.copy, nc.scalar.copy))
            for pt in range(2):
                for mix, msl in enumerate(MI):
                    evac_engs[pt][mix](out=g2b[:, pt, msl],
                                       in_=g2ps[pt][mix])

        # mu2 = colsum(V2)/D @ g2b + b2sum768 (+ colsum(x1full)/D if affine)
        ps_mu2 = big_ps("ps_mu2")
        for mix, msl in enumerate(MI):
            ops = [(wv2_b(r), g2b[:, r, msl]) for r in range(2)]
            if has_aff1:
                ops += [(ones_f, x1c[k][:, msl]) for k in range(KD)]
            for i, (lhsT, rhs) in enumerate(ops):
                mm(ps_mu2[:, msl], R(lhsT), R(rhs), start=(i == 0),
                   stop=(i == len(ops) - 1), skip_group_check=True)
        nc.vector.tensor_scalar_add(mu2_b, ps_mu2, b2sum768)

        # hoist the gelu->sqrt table swap off the LN2 tail
        nc.scalar.activation(out=lnv[:, 0:1], in_=ones_1[:, 0:1],
                             func=AF.Sqrt)

        # t2[k] = x1full[k] - mu2 (in place on x1c; b2 folds into z2c STT)
        for k in range(KD):
            eng = nc.gpsimd if k % 2 == 0 else nc.vector
            eng.tensor_tensor(out=x1c[k], in0=x1c[k], in1=mu2_b,
                              op=OP.subtract)

        # v2 + z2c + squares + var2 (var colsums lag v2 by 2)
        with tc.tile_pool(name="ps_st2", bufs=1, space="PSUM") as ps_st2:
            var2_ps = ps_st2.tile([P, M], f32, tag="stat", name="var2_ps")
            sq2s = {}

            def v2_k(k):
                ps = big_ps(f"ps_v2_{k}")
                for mix, msl in enumerate(MI):
                    for r in range(2):
                        mm(ps[:, msl], R(v2w[:, r, k * P:(k + 1) * P]),
                           R(g2b[:, r, msl]), start=(r == 0), stop=(r == 1),
                           skip_group_check=True)
                nc.vector.scalar_tensor_tensor(
                    out=xTt[:, k, :], in0=ps, scalar=b2c(k),
                    in1=x1c[k], op0=OP.add, op1=OP.add)
                sq2s[k] = sqp.tile([P, M], f32r, tag="sq", bufs=4,
                                   name=f"sq2_{k}")
                nc.scalar.activation(out=sq2s[k], in_=xTt[:, k, :],
                                     func=AF.Square)

            def var2_k(k):
                for mix, msl in enumerate(MI):
                    mm(var2_ps[:, msl], R(ones_1), R(sq2s[k][:, msl]),
                       start=(k == 0), stop=(k == KD - 1),
                       skip_group_check=True)

            LAG2 = 2
            for k in range(KD):
                v2_k(k)
                if k >= LAG2:
                    var2_k(k - LAG2)
            for k in range(KD - LAG2, KD):
                var2_k(k)

            nc.vector.reciprocal(out=lnv, in_=var2_ps)
            nc.scalar.activation(out=a2_b, in_=lnv, func=AF.Sqrt,
                                 scale=float(D))

        for k in range(KD):
            eng = nc.gpsimd if k in (0, 2) else nc.vector
            if has_aff2:
                eng.tensor_tensor(out=xTt[:, k, :], in0=xTt[:, k, :],
                                  in1=a2_b, op=OP.mult)
                nc.vector.tensor_scalar(
                    out=attn_sc[k], in0=xTt[:, k, :],
                    scalar1=affc[:, 12 + k:13 + k],
                    scalar2=affc[:, 18 + k:19 + k], op0=OP.mult, op1=OP.add)
            else:
                eng.tensor_tensor(out=attn_sc[k], in0=xTt[:, k, :],
                                  in1=a2_b, op=OP.mult)
            dma(out_d[k * P:(k + 1) * P, :], attn_sc[k])

    nc.compile()
    return nc


def _prep_inputs(x, mask, Pq, Vq, bq, Pk, Vk, bk, Pv, Vv, bv,
                 Uo, Vo, bo_attn, U1, V1, b1, U2, V2, b2,
                 ln1_g, ln1_b, ln2_g, ln2_b):
    """Host-side packing: per-core in_maps for the SPMD kernel."""
    has_aff1 = not (np.all(ln1_g == 1.0) and np.all(ln1_b == 0.0))
    has_aff2 = not (np.all(ln2_g == 1.0) and np.all(ln2_b == 0.0))

    # p_pack: group order [Q0 K0 V0 Q1 K1 V1 Q2 K2 V2]; 4 heads x 32/group
    p_pack = np.zeros((D, NG * P), np.float32)
    for t, Pw in enumerate((Pq, Pk, Pv)):
        for h in range(H):
            g = 3 * (h // 4) + t
            c0 = g * P + 32 * (h % 4)
            p_pack[:, c0:c0 + 32] = Pw[h]

    # w2pack: Q pairs | K pairs | V groups
    w2 = np.zeros((P, W_TOT), np.float32)
    for tt, Vw in ((0, Vq), (1, Vk)):
        for p in range(NPAIR):
            for s in range(2):
                h = 2 * p + s
                r0 = 32 * (h % 4)
                c0 = tt * NPAIR * P + p * P + 64 * s
                w2[r0:r0 + 32, c0:c0 + DH] = Vw[h]
    vbias = np.zeros((1, 3 * 4 * VW), np.float32)
    for h in range(H):
        vg, i = h // 4, h % 4
        w2[32 * i:32 * i + 32,
           W_V + vg * 4 * VW + VW * i:W_V + vg * 4 * VW + VW * i + DH] = Vv[h]
        vbias[0, vg * 4 * VW + VW * i:vg * 4 * VW + VW * i + DH] = \
            bv[0, h, 0, :]
        vbias[0, vg * 4 * VW + VW * i + DH] = 1.0

    # cpack [128, 56] fp32 (maskb filled per-core below)
    cpack = np.zeros((P, C_TOT), np.float32)
    cpack[:, C_BO:C_BO + KD] = np.asarray(bo_attn, np.float32).reshape(KD, P).T
    cpack[:, C_B2:C_B2 + KD] = np.asarray(b2, np.float32).reshape(KD, P).T
    cpack[:, C_B1:C_B1 + FFT] = np.asarray(b1, np.float32).reshape(FFT, P).T
    for p in range(NPAIR):
        cpack[0:DH, C_QKB + p] = bq[0, 2 * p, 0, :]
        cpack[DH:P, C_QKB + p] = bq[0, 2 * p + 1, 0, :]
        cpack[0:DH, C_QKB + NPAIR + p] = bk[0, 2 * p, 0, :]
        cpack[DH:P, C_QKB + NPAIR + p] = bk[0, 2 * p + 1, 0, :]

    def bcast_colsum(Vw):
        w = np.asarray(Vw, np.float32).sum(axis=1) / D   # [256]
        out = np.zeros((P, 2 * P), np.float32)
        for r in range(2):
            out[:, r * P:(r + 1) * P] = w[r * P:(r + 1) * P][:, None]
        return out

    wpack = np.ascontiguousarray(bcast_colsum(V2), np.float32)
    wvopack = bcast_colsum(Vo).astype(BF16)

    u1_eff = np.asarray(U1, np.float32)
    if has_aff1:
        u1_eff = np.asarray(ln1_g, np.float32)[:, None] * u1_eff

    def rpack(a, kd):
        a = np.asarray(a, np.float32)
        return a.reshape(kd, P, -1).transpose(1, 0, 2).reshape(P, -1)

    ovpack = np.concatenate(
        [rpack(Uo, KD), rpack(Vo, 2)], axis=1).astype(BF16)

    bosum768 = float(np.asarray(bo_attn, np.float32).sum() / D)
    b2sum768 = float(np.asarray(b2, np.float32).sum() / D)

    common = {
        "p_pack": p_pack.astype(BF16), "w2pack": w2.astype(BF16),
        "vbias": vbias, "wpack": wpack,
        "wvopack": wvopack,
        "ovpack": ovpack,
        "u1": np.ascontiguousarray(u1_eff, np.float32),
        "v1": np.asarray(V1, np.float32).astype(BF16),
        "u2": np.asarray(U2, np.float32).astype(BF16),
        "v2": np.ascontiguousarray(V2, np.float32),
    }
    if has_aff1 or has_aff2:
        affp = np.zeros((P, 26), np.float32)
        affp[:, 0:KD] = np.asarray(ln1_g, np.float32).reshape(KD, P).T
        affp[:, 6:6 + KD] = np.asarray(ln1_b, np.float32).reshape(KD, P).T
        affp[:, 12:12 + KD] = np.asarray(ln2_g, np.float32).reshape(KD, P).T
        affp[:, 18:18 + KD] = np.asarray(ln2_b, np.float32).reshape(KD, P).T
        mc1 = np.asarray(U1, np.float32).T @ np.asarray(ln1_b, np.float32)
        affp[:, 24:26] = mc1.reshape(2, P).T
        common["affpack"] = affp

    in_maps = []
    for b_i in range(B):
        m = dict(common)
        xt = np.ascontiguousarray(x[b_i].T, np.float32)
        m["xT"] = xt
        m["xb"] = xt.astype(BF16)
        cpk = cpack.copy()
        mb = np.where(mask[b_i] > 0, 0.0, -1e9).astype(np.float32)
        cpk[:, C_MASK:C_MASK + NPT] = mb.reshape(NPT, P).T
        m["cpack"] = cpk
        in_maps.append(m)
    return in_maps, has_aff1, has_aff2, bosum768, b2sum768


def build_program_for_inputs(**inputs):
    inputs = {k: np.asarray(v) for k, v in inputs.items()}
    in_maps, has_aff1, has_aff2, bosum768, b2sum768 = _prep_inputs(**inputs)
    key = (has_aff1, has_aff2, round(bosum768, 12), round(b2sum768, 12))
    if key not in _prog_cache:
        _prog_cache[key] = _build_program(has_aff1, has_aff2,
                                          bosum768, b2sum768)
    return _prog_cache[key], in_maps


def kernel(**inputs):
    global last_results
    nc, in_maps = build_program_for_inputs(**inputs)
    from concourse.bass_utils import run_bass_kernel_spmd
    res = run_bass_kernel_spmd(nc, in_maps, list(range(N_CORES)))
    last_results = res
    out = np.stack([np.asarray(res.results[b]["outT"], np.float32).T
                    for b in range(B)])
    return np.ascontiguousarray(out, np.float32)
